# revision 16
# baseline (speedup 1.0000x reference)
"""Trainium2 Bass kernel for nn_BinaryLayer: out = sign(x @ sign(W)).

x: [8192, 2048] f32, W: [2048, 2048] f32, out: [8192, 2048] f32 (values in {-1,0,1}).

Strategy: data-parallel batch shard across 8 cores (1024 rows each), W
replicated. Host does layout prep only (x shard transpose; W chunk reorder);
all binarization/conversion happens on device.

Default MODE "wstath" (measured ~144us/core, rel err 1.59e-2, 1054/16.7M sign
flips - deterministic for the fixed seed-0 inputs, gate is 2e-2):
  - W-STATIONARY fp16 layout: stationary operand = sign(W) chunk [128k,128n]
    fp16 (+-1/0 are fp16-exact, and 16-bit weights get the FWL fast path +
    background weight buffer, so LDWEIGHTS is fully hidden: measured 216ns
    per N=512 matmul vs 277ns for f32r, whose 4-byte weights can't
    double-buffer). Moving operand = x^T [128k, 512m] fp16 (f32 DMA ->
    VectorE convert; 11-bit mantissa is the sole error source). PSUM banks
    are out^T blocks [128n, 512m]; the host untransposes (free).
  - Everything resident in SBUF (W fp16 64KB/part + x fp16 32KB/part + f32
    staging): no batch halves, W+x each loaded exactly once (24MB/core).
  - Load stream in consumption order, alternating chunks between the sync
    and scalar HWDGE rings (aggregate ~320-375GB/s during the ramp). W
    Sign ops ride the scalar queue with a ~3-chunk lookahead; x converts on
    VectorE. Out^T is written bf16 (sign values exact, half the traffic) via
    the gpsimd SWDGE ring; the final sweep's outs use the scalar ring (fast
    completion) to shrink the drain tail.
  - Compute: first sweep = 8 psum banks (j0-3 x both m-halves), k-outer so
    the PE consumes x/W chunks as they land (the 12MB ramp bounds this
    phase); then 4-bank sweeps alternating psum-pool halves so evictions
    (VectorE (psum>0)-(psum<0) 2-op, or ScalarE Sign) fully overlap the next
    sweep; the last sweep is bank-serial so the post-last-matmul drain is a
    single bank. ~9us framework preamble, ~3us barrier tail.

Other modes kept for reference/fallback:
  "wstatf32r" - same structure, both operands float32r (FP22): ~158us,
            rel err 1.13e-2 (536 flips). The extra ~60 cycles/matmul is
            unhidden f32r LDWEIGHTS.
  "hilo2" - original x-stationary 2-pass bf16 hi/lo, near-fp32-exact
            (1.8e-3), ~250us. Use if the tolerance ever tightens.
  "f32r1" - original x-stationary 1-pass f32r, ~173us, 1.13e-2.
  "wstat" (bf16 W x f32r x) is rejected by walrus ("Mixing of 32-bit and
  non-32-bit Matmult inputs"); "wstatf32rl" (ldweights=False on the second
  matmul of each stationary pair) computes WRONG results - do not use.
"""

import numpy as np

B, D_IN, D_OUT = 8192, 2048, 2048
N_CORES = 8
BS = B // N_CORES  # 1024 batch rows per core
P = 128
KT = D_IN // P  # 16 k-tiles
NCH = 512  # psum bank width (f32)
NT = D_OUT // NCH  # 4 n-chunks

MODE = "wstath"

_CACHE: dict = {}


def build_bass(mode: str = MODE):
    import concourse.mybir as mybir
    import concourse.tile as tile
    from concourse import bacc
    from contextlib import ExitStack

    f32 = mybir.dt.float32
    bf16 = mybir.dt.bfloat16
    f32r = mybir.dt.float32r
    Sign = mybir.ActivationFunctionType.Sign

    # Bacc (not plain Bass): its finalize() runs move_matmul_waits_to_ldweights
    # + generate_event_semaphores, which legalize multi-wait instructions for
    # walrus (each non-event instruction may carry at most one sync wait).
    nc = bacc.Bacc()
    xT = nc.declare_dram_parameter("xT", [D_IN, BS], f32, isOutput=False)
    if mode.startswith("wstat"):
        # W relaid on host into (quarter, k-tile) stream order: chunk (q,k)
        # = W[k*128:(k+1)*128, q*512:(q+1)*512], 256KB contiguous each.
        w = nc.declare_dram_parameter("w", [4 * KT * P, NCH], f32, isOutput=False)
        # out^T in bf16: sign values {-1,0,+1} are bf16-exact; halves the
        # outbound traffic. Host untransposes + converts.
        out = nc.declare_dram_parameter("out", [D_OUT, BS], mybir.dt.bfloat16, isOutput=True)
    else:
        w = nc.declare_dram_parameter("w", [D_IN, D_OUT], f32, isOutput=False)
        out = nc.declare_dram_parameter("out", [BS, D_OUT], f32, isOutput=True)

    with ExitStack() as ctx:
        tc = ctx.enter_context(tile.TileContext(nc))
        res_pool = ctx.enter_context(tc.tile_pool(name="resident", bufs=1))
        xstage = ctx.enter_context(tc.tile_pool(name="xstage", bufs=2))
        # wstatf32r keeps W resident as f32r (128KB/part) - staging pools
        # must shrink to fit the ~208KB/part SBUF budget.
        wstage = ctx.enter_context(
            tc.tile_pool(name="wstage", bufs=3 if mode.startswith("wstatf32r") else 8)
        )
        psum_pool = ctx.enter_context(tc.tile_pool(name="psum", bufs=8, space="PSUM"))
        ostage = ctx.enter_context(
            tc.tile_pool(name="ostage", bufs=3 if mode.startswith("wstatf32r") else 8)
        )

        # W is loaded in half-rows [128, 1024] (4KB contiguous per partition
        # row — 2KB-run column chunks measured only ~225GB/s vs ~300GB/s).
        # f32r note: walrus's verifier requires every writer of an FP32r
        # matmul operand to itself produce float32r, so the f32r tiles are
        # declared f32r, DMAs bitcast the DRAM side (pure byte copy), and the
        # in-place Sign writes f32r (+-1.0 is FP22-exact).
        WH = NCH * 2  # 1024: W half-row width
        NH = D_OUT // WH  # 2 halves
        wdt = bf16 if mode == "hilo2" else f32r
        wbin = [] if mode.startswith("wstat") else [
            [
                res_pool.tile([P, WH], wdt, tag=f"wb{k}_{h}", name=f"wb{k}_{h}")
                for h in range(NH)
            ]
            for k in range(KT)
        ]

        NPH = WH // NCH  # n-chunks per W half

        def wbin_slice(k, n):
            return wbin[k][n // NPH][:, (n % NPH) * NCH : (n % NPH + 1) * NCH]

        def load_w_half(k, h, split=False):
            wsl = w[k * P : (k + 1) * P, h * WH : (h + 1) * WH]
            if mode == "hilo2":
                w32 = xstage.tile([P, WH], f32, tag="w32", name="w32", bufs=3)
                if split:
                    # Two pieces so the first matmul's rhs dep lands sooner.
                    for a, b in ((0, WH // 2), (WH // 2, WH)):
                        nc.sync.dma_start(w32[:, a:b], wsl[:, a:b])
                        nc.scalar.activation(
                            wbin[k][h][:, a:b], w32[:, a:b], Sign
                        )
                else:
                    nc.sync.dma_start(w32[:], wsl)
                    nc.scalar.activation(wbin[k][h][:], w32[:], Sign)
            else:
                # Load into the resident f32r tile and binarize in place.
                nc.sync.dma_start(wbin[k][h][:], wsl.bitcast(f32r))
                nc.scalar.activation(
                    wbin[k][h][:], wbin[k][h][:].bitcast(f32), Sign
                )

        if mode == "hilo2":
            MT = BS // P  # 8 m-tiles
            xhi = [
                res_pool.tile([P, BS], bf16, tag=f"xhi{k}", name=f"xhi{k}")
                for k in range(KT)
            ]
            xlo = [
                res_pool.tile([P, BS], bf16, tag=f"xlo{k}", name=f"xlo{k}")
                for k in range(KT)
            ]

            # Stream: x k-tiles + the first W halves, then the second halves.
            # k=0 is loaded/split in two column pieces so the first matmul's
            # dependencies (xhi[0][:, :128], wbin[0][0][:, :512]) land fast.
            for k in range(KT):
                x32 = xstage.tile([P, BS], f32, tag="x32", name="x32")
                if k == 0 and BS > P:
                    # First-matmul critical path: tiny x piece, then tiny W
                    # piece, before the remainders (queue order = issue order).
                    nc.sync.dma_start(x32[:, :P], xT[0:P, 0:P])
                    nc.vector.tensor_copy(xhi[0][:, :P], x32[:, :P])
                    nc.vector.tensor_sub(xlo[0][:, :P], x32[:, :P], xhi[0][:, :P])
                    load_w_half(k, 0, split=True)
                    nc.sync.dma_start(x32[:, P:], xT[0:P, P:BS])
                    nc.vector.tensor_copy(xhi[0][:, P:], x32[:, P:])
                    nc.vector.tensor_sub(xlo[0][:, P:], x32[:, P:], xhi[0][:, P:])
                else:
                    nc.sync.dma_start(x32[:], xT[k * P : (k + 1) * P, :])
                    nc.vector.tensor_copy(xhi[k][:], x32[:])
                    nc.vector.tensor_sub(xlo[k][:], x32[:], xhi[k][:])
                    load_w_half(k, 0)
            for h in range(1, NH):
                for k in range(KT):
                    load_w_half(k, h)

            # Process n-chunks in pairs (4 m-tiles x 2 n-chunks = 8 PSUM
            # banks): the first pair consumes only W half 0, giving the
            # half-1 DMA stream until ~t=115us to land instead of ~66us.
            # The FIRST sweep is k-outer (consumes W/x k-tiles as they
            # stream); later sweeps are m-outer/k-inner so each PSUM bank
            # completes and evicts individually - the next sweep's matmuls
            # start as soon as a bank frees instead of stalling on a bulk
            # eviction boundary.
            NP = 2  # n-chunks per pair
            MQ = MT // 2  # m-tiles processed per pair sweep (4)

            def evict(psum, m, n, use_act=False):
                # sign(psum) on VectorE as (psum>0) - (psum<0): keeps the
                # eviction off ScalarE, whose in-order queue still holds
                # W-half-1 Sign ops that wait on their DMAs (head-of-line
                # blocking stalled the PE for ~6us at the first sweep edge).
                # The last pair alternates onto ScalarE (idle by then) so the
                # post-last-matmul eviction drain is shorter.
                ot = ostage.tile([P, NCH], f32, tag="ot", name="ot")
                if use_act:
                    nc.scalar.activation(ot[:], psum[:], Sign)
                else:
                    lt = ostage.tile([P, NCH], f32, tag="lt", name="lt")
                    nc.vector.tensor_scalar(
                        lt[:], psum[:], 0.0, None, mybir.AluOpType.is_lt
                    )
                    nc.vector.scalar_tensor_tensor(
                        ot[:],
                        psum[:],
                        0.0,
                        lt[:],
                        op0=mybir.AluOpType.is_gt,
                        op1=mybir.AluOpType.subtract,
                    )
                nc.sync.dma_start(
                    out[m * P : (m + 1) * P, n * NCH : (n + 1) * NCH], ot[:]
                )

            first = True
            for np_ in range(NT // NP):
                for mh in range(2):
                    if first:
                        first = False
                        psums = [
                            [
                                psum_pool.tile([P, NCH], f32, tag="ps", name="ps")
                                for _ in range(NP)
                            ]
                            for _ in range(MQ)
                        ]
                        for k in range(KT):
                            for pi, src in enumerate((xhi, xlo)):
                                for mi in range(MQ):
                                    m = mh * MQ + mi
                                    for ni in range(NP):
                                        nc.tensor.matmul(
                                            psums[mi][ni][:],
                                            src[k][:, m * P : (m + 1) * P],
                                            wbin_slice(k, np_ * NP + ni),
                                            start=(k == 0 and pi == 0),
                                            stop=(k == KT - 1 and pi == 1),
                                        )
                        for mi in range(MQ):
                            for ni in range(NP):
                                evict(
                                    psums[mi][ni],
                                    mh * MQ + mi,
                                    np_ * NP + ni,
                                )
                    else:
                        for mi in range(MQ):
                            m = mh * MQ + mi
                            for ni in range(NP):
                                n = np_ * NP + ni
                                ps = psum_pool.tile(
                                    [P, NCH], f32, tag="ps", name="ps"
                                )
                                for k in range(KT):
                                    for pi, src in enumerate((xhi, xlo)):
                                        nc.tensor.matmul(
                                            ps[:],
                                            src[k][:, m * P : (m + 1) * P],
                                            wbin_slice(k, n),
                                            start=(k == 0 and pi == 0),
                                            stop=(k == KT - 1 and pi == 1),
                                        )
                                evict(
                                    ps,
                                    m,
                                    n,
                                    use_act=(
                                        np_ == NT // NP - 1
                                        and (mi * NP + ni) % 2 == 1
                                    ),
                                )

        elif mode.startswith("wstat"):
            # W-STATIONARY layout: stationary = sign(W) chunk [128k, 128n],
            # moving = x^T [128k, 512m] f32r, psum = out^T block [128n, 512m].
            #
            # Why: the f32r1 trace shows a 277ns/matmul pace = 512 (moving
            # cols) + ~133 cycles of unhidden LDWEIGHTS - fp32-family
            # stationary operands get no FWL and (4-byte weights) no
            # background weight buffer. sign(W) is {-1,0,+1} = EXACTLY
            # representable in bf16, so a bf16 stationary W restores the
            # hidden-LDWEIGHTS fast path (pace ~= 512 cycles = 220ns) while
            # x stays f32r (fp22) for precision: same error as f32r1.
            # "wstatf32r" keeps W f32r (fallback if mixed dtypes fail).
            f16 = mybir.dt.float16
            wdt = {"wstat": bf16, "wstath": f16}.get(mode, f32r)
            xdt = f16 if mode == "wstath" else f32r
            skip_ldw = mode == "wstatf32rl"
            JT = D_OUT // P  # 16 stationary j-tiles (128 out^T rows each)
            MH = BS // NCH  # 2 moving halves
            xres = [
                res_pool.tile([P, BS], xdt, tag=f"xr{k}", name=f"xr{k}")
                for k in range(KT)
            ]
            wres = [
                res_pool.tile([P, D_OUT], wdt, tag=f"wr{k}", name=f"wr{k}")
                for k in range(KT)
            ]

            # --- load stream, in consumption order, alternating between the
            # sync and scalar HWDGE rings so the ordered stream gets both
            # rings' aggregate bandwidth (~358GB/s HBM cap) instead of one.
            # W chunk (q,k) = sign-source for wres[k][:, q*512:(q+1)*512],
            # 256KB contiguous in the host-relaid wl. Sign ops ride the
            # scalar queue with ~3-chunk lookahead so DMA dispatches stay
            # ahead of the compute waits (no head-of-line starvation).
            rings = [nc.sync, nc.scalar]
            state = {"xring": 1, "wring": 0}
            pend: list = []

            def ring(kind):
                r = rings[state[kind] % 2]
                state[kind] += 1
                return r

            def sign_one():
                q, k, st = pend.pop(0)
                nc.scalar.activation(
                    wres[k][:, q * NCH : (q + 1) * NCH], st[:], Sign
                )

            def load_w(q, k, lag=3, split=False):
                st = wstage.tile([P, NCH], f32, tag="wst", name="wst")
                r0 = (q * KT + k) * P
                if split:
                    # ramp-critical chunks: two contiguous row-halves land on
                    # both rings simultaneously (no per-k arrival skew); the
                    # Sign reads the whole tile so it waits for both.
                    HP = P // 2
                    nc.sync.dma_start(st[:HP, :], w[r0 : r0 + HP, :])
                    nc.scalar.dma_start(st[HP:, :], w[r0 + HP : r0 + P, :])
                else:
                    ring("wring").dma_start(st[:], w[r0 : r0 + P, :])
                pend.append((q, k, st))
                while len(pend) > lag:
                    sign_one()

            xcvt = {}

            def load_x(k, a, b):
                if mode == "wstath":
                    # f32 bytes -> staging -> DVE convert (round-to-nearest)
                    # to the fp16 resident tile. Row-halves ride both rings
                    # (contiguous 256KB each, no skew); the convert waits for
                    # both.
                    if k not in xcvt:
                        xcvt[k] = xstage.tile(
                            [P, BS], f32, tag="xcv", name="xcv", bufs=3
                        )
                    st = xcvt[k]
                    ring("xring").dma_start(st[:, a:b], xT[k * P : (k + 1) * P, a:b])
                    nc.vector.tensor_copy(xres[k][:, a:b], st[:, a:b])
                else:
                    ring("xring").dma_start(
                        xres[k][:, a:b],
                        xT[k * P : (k + 1) * P, a:b].bitcast(f32r),
                    )

            # k=0 split small so the first matmul's deps land fast; the
            # first W chunk leads on the sync ring while x(0) leads on the
            # scalar ring. Sign ops share the scalar queue with half the DMA
            # dispatches, and a Sign waiting on an un-landed chunk blocks
            # every later dispatch behind it (in-order queue) - so signs run
            # with a deep lookahead (lag 6 <= wstage bufs-2) so they only
            # ever touch chunks that have already landed. The first two W
            # chunks use small lags to unblock the first matmuls, emitted
            # after the early x dispatches.
            load_w(0, 0, lag=99)
            load_x(0, 0, NCH)
            sign_one()  # sign(q0,k0): x(0a) dispatch already queued ahead
            load_x(0, NCH, BS)
            for k in range(1, KT):
                # two column-half pieces land on both rings ~simultaneously
                # (row-half splits were tried: a 64-partition DMA only fans
                # out to half the SDMA engines and measured ~18us slower).
                load_x(k, 0, NCH)
                load_x(k, NCH, BS)
                load_w(0, k, lag=1 if k < 4 else 3)
            for q in range(1, 4):
                for k in range(KT):
                    load_w(q, k, lag=3)
            while pend:
                sign_one()

            # --- compute sweeps. First sweep: 8 banks (j0-3 x mh0-1),
            # k-outer so the PE consumes x/W chunks as they land. Later
            # sweeps: 4 banks (j-pair x mh), alternating between the two
            # halves of the PSUM pool so sweep s+2 reuses banks freed by
            # sweep s - eviction (VectorE) fully overlaps the next sweep.
            def mm(ps, k, j, mh, start, stop):
                r = nc.tensor.matmul(
                    ps[:],
                    wres[k][:, j * P : (j + 1) * P],
                    xres[k][:, mh * NCH : (mh + 1) * NCH],
                    start=start,
                    stop=stop,
                )
                if skip_ldw and mh > 0:
                    # Same stationary W[k][j] as the mh-1 matmul: skip the
                    # redundant (unhidden, ~133-cycle for f32r) LDWEIGHTS.
                    r.ins.ldweights = False

            def evict(ps, j, mh, use_act=False, out_ring=None):
                ot = ostage.tile([P, NCH], bf16, tag="ot", name="ot")
                if use_act:
                    nc.scalar.activation(ot[:], ps[:], Sign)
                else:
                    lt = ostage.tile([P, NCH], bf16, tag="lt", name="lt")
                    nc.vector.tensor_scalar(
                        lt[:], ps[:], 0.0, None, mybir.AluOpType.is_lt
                    )
                    nc.vector.scalar_tensor_tensor(
                        ot[:],
                        ps[:],
                        0.0,
                        lt[:],
                        op0=mybir.AluOpType.is_gt,
                        op1=mybir.AluOpType.subtract,
                    )
                (out_ring or nc.gpsimd).dma_start(
                    out[j * P : (j + 1) * P, mh * NCH : (mh + 1) * NCH], ot[:]
                )

            def bank():
                return psum_pool.tile([P, NCH], f32, tag="ps", name="ps")

            banks = [[bank() for _ in range(MH)] for _ in range(4)]
            for k in range(KT):
                for j in range(4):
                    for mh in range(MH):
                        mm(banks[j][mh], k, j, mh, k == 0, k == KT - 1)
            for j in range(4):
                for mh in range(MH):
                    evict(banks[j][mh], j, mh)

            pairs = [(4, 5), (6, 7), (8, 9), (10, 11), (12, 13)]
            for t, jp in enumerate(pairs):
                banks = [[bank() for _ in range(MH)] for _ in range(2)]
                for k in range(KT):
                    for ji in range(2):
                        for mh in range(MH):
                            mm(banks[ji][mh], k, jp[ji], mh, k == 0, k == KT - 1)
                for ji in range(2):
                    for mh in range(MH):
                        # late sweeps: ScalarE (idle by then) takes half the
                        # evictions to shorten the drain tail.
                        evict(
                            banks[ji][mh],
                            jp[ji],
                            mh,
                            use_act=(t >= 3 and (ji * MH + mh) % 2 == 1),
                        )
            # Final sweep (j14,j15): W is fully resident by now, so run
            # bank-serial k-inner - each bank completes every 16 matmuls and
            # its eviction/store overlaps the next bank's matmuls, shrinking
            # the post-last-matmul drain to one bank. Outs ride the scalar
            # HWDGE ring (its loads have long drained; ~0.6us completion vs
            # ~2-3us SWDGE).
            ei = 0
            for j in (14, 15):
                for mh in range(MH):
                    ps = bank()
                    for k in range(KT):
                        mm(ps, k, j, mh, k == 0, k == KT - 1)
                    evict(ps, j, mh, use_act=(ei % 2 == 1), out_ring=nc.scalar)
                    ei += 1

        elif mode == "f32r1":
            NBH = 2  # batch halves (SBUF: W f32 128KB/part + x half 32KB/part)
            BS2 = BS // NBH  # 512
            MT2 = BS2 // P  # 4 m-tiles per half
            xres = [
                res_pool.tile([P, BS2], f32r, tag=f"xr{k}", name=f"xr{k}")
                for k in range(KT)
            ]

            def load_x(k, bh):
                # Direct byte-copy into the f32r tile; the PE truncates fp32
                # to FP22 on read. (A DVE fp32->f32r staging copy was tried:
                # bit-identical flips - DVE truncates too - and it slowed the
                # stream by ~25us. Reverted.)
                nc.sync.dma_start(
                    xres[k][:],
                    xT[k * P : (k + 1) * P, bh * BS2 : (bh + 1) * BS2].bitcast(
                        f32r
                    ),
                )

            for bh in range(NBH):
                for k in range(KT):
                    load_x(k, bh)
                    if bh == 0:
                        # First half: interleave x with the first W halves.
                        load_w_half(k, 0)
                if bh == 0:
                    for h in range(1, NH):
                        for k in range(KT):
                            load_w_half(k, h)

                for n in range(NT):
                    psums = [
                        psum_pool.tile([P, NCH], f32, tag="ps", name="ps")
                        for _ in range(MT2)
                    ]
                    for k in range(KT):
                        for m in range(MT2):
                            nc.tensor.matmul(
                                psums[m][:],
                                xres[k][:, m * P : (m + 1) * P],
                                wbin_slice(k, n),
                                start=(k == 0),
                                stop=(k == KT - 1),
                            )
                    for m in range(MT2):
                        ot = ostage.tile([P, NCH], f32, tag="ot", name="ot")
                        nc.scalar.activation(ot[:], psums[m][:], Sign)
                        nc.sync.dma_start(
                            out[
                                bh * BS2 + m * P : bh * BS2 + (m + 1) * P,
                                n * NCH : (n + 1) * NCH,
                            ],
                            ot[:],
                        )
        else:
            raise ValueError(mode)

    nc.finalize()
    return nc


def _shard_inputs(x: np.ndarray, kernel: np.ndarray, mode: str = MODE):
    """Per-core input maps: batch-shard x (pre-transposed layout), replicate W.

    Pure layout prep only (transpose/reorder of raw f32 bytes) - all
    binarization/compute happens on device.
    """
    if mode.startswith("wstat"):
        # (quarter, k-tile) stream-ordered relayout of the replicated W.
        wl = np.concatenate(
            [
                kernel[k * P : (k + 1) * P, q * NCH : (q + 1) * NCH]
                for q in range(4)
                for k in range(KT)
            ],
            axis=0,
        )
        wl = np.ascontiguousarray(wl)
    else:
        wl = kernel
    in_maps = []
    for i in range(N_CORES):
        xs = np.ascontiguousarray(x[i * BS : (i + 1) * BS, :].T)
        in_maps.append({"xT": xs, "w": wl})
    return in_maps


def run_on_cores(x: np.ndarray, kernel: np.ndarray, mode: str = MODE, **run_kwargs):
    """Compile (cached) and run the SPMD kernel; returns (full_out, BassKernelResults)."""
    from concourse.bass_utils import run_bass_kernel_spmd

    key = ("nc", mode)
    if key not in _CACHE:
        _CACHE[key] = build_bass(mode)
    nc = _CACHE[key]

    in_maps = _shard_inputs(x, kernel, mode)
    res = run_bass_kernel_spmd(nc, in_maps, list(range(N_CORES)), **run_kwargs)
    if mode.startswith("wstat"):
        # Per-core result is out^T [D_OUT, BS] bf16: untranspose + upcast.
        out = np.concatenate(
            [
                np.asarray(res.results[i]["out"]).astype(np.float32).T
                for i in range(N_CORES)
            ],
            axis=0,
        )
    else:
        out = np.concatenate(
            [res.results[i]["out"] for i in range(N_CORES)], axis=0
        )
    return out, res


def kernel(x: np.ndarray, kernel: np.ndarray) -> np.ndarray:
    assert x.shape == (B, D_IN) and kernel.shape == (D_IN, D_OUT)
    out, _ = run_on_cores(
        np.asarray(x, dtype=np.float32), np.asarray(kernel, dtype=np.float32)
    )
    return out.astype(np.float32)



# revision 17
# speedup vs baseline: 1.1564x; 1.1564x over previous
"""Trainium2 Bass kernel for nn_BinaryLayer: out = sign(x @ sign(W)).

x: [8192, 2048] f32, W: [2048, 2048] f32, out: [8192, 2048] f32 (values in {-1,0,1}).

Strategy: data-parallel batch shard across 8 cores (1024 rows each), W
replicated. Host does layout prep only (x shard transpose; W chunk reorder);
all binarization/conversion happens on device.

Default MODE "wstath" (measured 142.6-147us/core across runs - device-level
run-to-run variance of several us was observed, with occasional ~10% slower
outliers under repeated back-to-back runs (thermal/SW throttle); rel err
1.585e-2, 1054/16.7M sign flips - deterministic for the fixed seed-0 inputs,
gate is 2e-2):
  - W-STATIONARY fp16 layout: stationary operand = sign(W) chunk [128k,128n]
    fp16 (+-1/0 are fp16-exact, and 16-bit weights get the FWL fast path +
    background weight buffer, so LDWEIGHTS is fully hidden: measured 216ns
    per N=512 matmul vs 277ns for f32r, whose 4-byte weights can't
    double-buffer). Moving operand = x^T [128k, 512m] fp16 (f32 DMA ->
    VectorE convert; 11-bit mantissa is the sole error source). PSUM banks
    are out^T blocks [128n, 512m]; the host untransposes (free).
  - Everything resident in SBUF (W fp16 64KB/part + x fp16 32KB/part + f32
    staging): no batch halves, W+x each loaded exactly once (24MB/core).
  - Load stream in consumption order, alternating chunks between the sync
    and scalar HWDGE rings (aggregate ~320-375GB/s during the ramp). W
    Sign ops ride the scalar queue with a ~3-chunk lookahead; x converts on
    VectorE. Out^T is written bf16 (sign values exact, half the traffic) via
    the gpsimd SWDGE ring; the final sweep's outs use the scalar ring (fast
    completion) to shrink the drain tail.
  - Compute: first sweep = 8 psum banks (j0-3 x both m-halves), k-outer so
    the PE consumes x/W chunks as they land (the 12MB ramp bounds this
    phase); then 4-bank sweeps alternating psum-pool halves so evictions
    (VectorE (psum>0)-(psum<0) 2-op, or ScalarE Sign) fully overlap the next
    sweep; the last sweep is bank-serial so the post-last-matmul drain is a
    single bank. ~9us framework preamble, ~3us barrier tail.

Other modes kept for reference/fallback:
  "wstatf32r" - same structure, both operands float32r (FP22): ~158us,
            rel err 1.13e-2 (536 flips). The extra ~60 cycles/matmul is
            unhidden f32r LDWEIGHTS.
  "hilo2" - original x-stationary 2-pass bf16 hi/lo, near-fp32-exact
            (1.8e-3), ~250us. Use if the tolerance ever tightens.
  "f32r1" - original x-stationary 1-pass f32r, ~173us, 1.13e-2.
  "wstat" (bf16 W x f32r x) is rejected by walrus ("Mixing of 32-bit and
  non-32-bit Matmult inputs"); "wstatf32rl" (ldweights=False on the second
  matmul of each stationary pair) computes WRONG results - do not use.
"""

import numpy as np

B, D_IN, D_OUT = 8192, 2048, 2048
N_CORES = 8
BS = B // N_CORES  # 1024 batch rows per core
P = 128
KT = D_IN // P  # 16 k-tiles
NCH = 512  # psum bank width (f32)
NT = D_OUT // NCH  # 4 n-chunks

MODE = "wstath"

_CACHE: dict = {}


def build_bass(mode: str = MODE):
    import concourse.mybir as mybir
    import concourse.tile as tile
    from concourse import bacc
    from contextlib import ExitStack

    f32 = mybir.dt.float32
    bf16 = mybir.dt.bfloat16
    f32r = mybir.dt.float32r
    Sign = mybir.ActivationFunctionType.Sign

    # Bacc (not plain Bass): its finalize() runs move_matmul_waits_to_ldweights
    # + generate_event_semaphores, which legalize multi-wait instructions for
    # walrus (each non-event instruction may carry at most one sync wait).
    nc = bacc.Bacc()
    xT = nc.declare_dram_parameter("xT", [D_IN, BS], f32, isOutput=False)
    if mode.startswith("wstat"):
        # W relaid on host into (quarter, k-tile) stream order: chunk (q,k)
        # = W[k*128:(k+1)*128, q*512:(q+1)*512], 256KB contiguous each.
        w = nc.declare_dram_parameter("w", [4 * KT * P, NCH], f32, isOutput=False)
        # out^T in bf16: sign values {-1,0,+1} are bf16-exact; halves the
        # outbound traffic. Host untransposes + converts.
        out = nc.declare_dram_parameter("out", [D_OUT, BS], mybir.dt.bfloat16, isOutput=True)
    else:
        w = nc.declare_dram_parameter("w", [D_IN, D_OUT], f32, isOutput=False)
        out = nc.declare_dram_parameter("out", [BS, D_OUT], f32, isOutput=True)

    with ExitStack() as ctx:
        tc = ctx.enter_context(tile.TileContext(nc))
        res_pool = ctx.enter_context(tc.tile_pool(name="resident", bufs=1))
        xstage = ctx.enter_context(tc.tile_pool(name="xstage", bufs=2))
        # wstatf32r keeps W resident as f32r (128KB/part) - staging pools
        # must shrink to fit the ~208KB/part SBUF budget.
        wstage = ctx.enter_context(
            tc.tile_pool(name="wstage", bufs=3 if mode.startswith("wstatf32r") else 8)
        )
        psum_pool = ctx.enter_context(tc.tile_pool(name="psum", bufs=8, space="PSUM"))
        ostage = ctx.enter_context(
            tc.tile_pool(name="ostage", bufs=3 if mode.startswith("wstatf32r") else 8)
        )

        # W is loaded in half-rows [128, 1024] (4KB contiguous per partition
        # row — 2KB-run column chunks measured only ~225GB/s vs ~300GB/s).
        # f32r note: walrus's verifier requires every writer of an FP32r
        # matmul operand to itself produce float32r, so the f32r tiles are
        # declared f32r, DMAs bitcast the DRAM side (pure byte copy), and the
        # in-place Sign writes f32r (+-1.0 is FP22-exact).
        WH = NCH * 2  # 1024: W half-row width
        NH = D_OUT // WH  # 2 halves
        wdt = bf16 if mode == "hilo2" else f32r
        wbin = [] if mode.startswith("wstat") else [
            [
                res_pool.tile([P, WH], wdt, tag=f"wb{k}_{h}", name=f"wb{k}_{h}")
                for h in range(NH)
            ]
            for k in range(KT)
        ]

        NPH = WH // NCH  # n-chunks per W half

        def wbin_slice(k, n):
            return wbin[k][n // NPH][:, (n % NPH) * NCH : (n % NPH + 1) * NCH]

        def load_w_half(k, h, split=False):
            wsl = w[k * P : (k + 1) * P, h * WH : (h + 1) * WH]
            if mode == "hilo2":
                w32 = xstage.tile([P, WH], f32, tag="w32", name="w32", bufs=3)
                if split:
                    # Two pieces so the first matmul's rhs dep lands sooner.
                    for a, b in ((0, WH // 2), (WH // 2, WH)):
                        nc.sync.dma_start(w32[:, a:b], wsl[:, a:b])
                        nc.scalar.activation(
                            wbin[k][h][:, a:b], w32[:, a:b], Sign
                        )
                else:
                    nc.sync.dma_start(w32[:], wsl)
                    nc.scalar.activation(wbin[k][h][:], w32[:], Sign)
            else:
                # Load into the resident f32r tile and binarize in place.
                nc.sync.dma_start(wbin[k][h][:], wsl.bitcast(f32r))
                nc.scalar.activation(
                    wbin[k][h][:], wbin[k][h][:].bitcast(f32), Sign
                )

        if mode == "hilo2":
            MT = BS // P  # 8 m-tiles
            xhi = [
                res_pool.tile([P, BS], bf16, tag=f"xhi{k}", name=f"xhi{k}")
                for k in range(KT)
            ]
            xlo = [
                res_pool.tile([P, BS], bf16, tag=f"xlo{k}", name=f"xlo{k}")
                for k in range(KT)
            ]

            # Stream: x k-tiles + the first W halves, then the second halves.
            # k=0 is loaded/split in two column pieces so the first matmul's
            # dependencies (xhi[0][:, :128], wbin[0][0][:, :512]) land fast.
            for k in range(KT):
                x32 = xstage.tile([P, BS], f32, tag="x32", name="x32")
                if k == 0 and BS > P:
                    # First-matmul critical path: tiny x piece, then tiny W
                    # piece, before the remainders (queue order = issue order).
                    nc.sync.dma_start(x32[:, :P], xT[0:P, 0:P])
                    nc.vector.tensor_copy(xhi[0][:, :P], x32[:, :P])
                    nc.vector.tensor_sub(xlo[0][:, :P], x32[:, :P], xhi[0][:, :P])
                    load_w_half(k, 0, split=True)
                    nc.sync.dma_start(x32[:, P:], xT[0:P, P:BS])
                    nc.vector.tensor_copy(xhi[0][:, P:], x32[:, P:])
                    nc.vector.tensor_sub(xlo[0][:, P:], x32[:, P:], xhi[0][:, P:])
                else:
                    nc.sync.dma_start(x32[:], xT[k * P : (k + 1) * P, :])
                    nc.vector.tensor_copy(xhi[k][:], x32[:])
                    nc.vector.tensor_sub(xlo[k][:], x32[:], xhi[k][:])
                    load_w_half(k, 0)
            for h in range(1, NH):
                for k in range(KT):
                    load_w_half(k, h)

            # Process n-chunks in pairs (4 m-tiles x 2 n-chunks = 8 PSUM
            # banks): the first pair consumes only W half 0, giving the
            # half-1 DMA stream until ~t=115us to land instead of ~66us.
            # The FIRST sweep is k-outer (consumes W/x k-tiles as they
            # stream); later sweeps are m-outer/k-inner so each PSUM bank
            # completes and evicts individually - the next sweep's matmuls
            # start as soon as a bank frees instead of stalling on a bulk
            # eviction boundary.
            NP = 2  # n-chunks per pair
            MQ = MT // 2  # m-tiles processed per pair sweep (4)

            def evict(psum, m, n, use_act=False):
                # sign(psum) on VectorE as (psum>0) - (psum<0): keeps the
                # eviction off ScalarE, whose in-order queue still holds
                # W-half-1 Sign ops that wait on their DMAs (head-of-line
                # blocking stalled the PE for ~6us at the first sweep edge).
                # The last pair alternates onto ScalarE (idle by then) so the
                # post-last-matmul eviction drain is shorter.
                ot = ostage.tile([P, NCH], f32, tag="ot", name="ot")
                if use_act:
                    nc.scalar.activation(ot[:], psum[:], Sign)
                else:
                    lt = ostage.tile([P, NCH], f32, tag="lt", name="lt")
                    nc.vector.tensor_scalar(
                        lt[:], psum[:], 0.0, None, mybir.AluOpType.is_lt
                    )
                    nc.vector.scalar_tensor_tensor(
                        ot[:],
                        psum[:],
                        0.0,
                        lt[:],
                        op0=mybir.AluOpType.is_gt,
                        op1=mybir.AluOpType.subtract,
                    )
                nc.sync.dma_start(
                    out[m * P : (m + 1) * P, n * NCH : (n + 1) * NCH], ot[:]
                )

            first = True
            for np_ in range(NT // NP):
                for mh in range(2):
                    if first:
                        first = False
                        psums = [
                            [
                                psum_pool.tile([P, NCH], f32, tag="ps", name="ps")
                                for _ in range(NP)
                            ]
                            for _ in range(MQ)
                        ]
                        for k in range(KT):
                            for pi, src in enumerate((xhi, xlo)):
                                for mi in range(MQ):
                                    m = mh * MQ + mi
                                    for ni in range(NP):
                                        nc.tensor.matmul(
                                            psums[mi][ni][:],
                                            src[k][:, m * P : (m + 1) * P],
                                            wbin_slice(k, np_ * NP + ni),
                                            start=(k == 0 and pi == 0),
                                            stop=(k == KT - 1 and pi == 1),
                                        )
                        for mi in range(MQ):
                            for ni in range(NP):
                                evict(
                                    psums[mi][ni],
                                    mh * MQ + mi,
                                    np_ * NP + ni,
                                )
                    else:
                        for mi in range(MQ):
                            m = mh * MQ + mi
                            for ni in range(NP):
                                n = np_ * NP + ni
                                ps = psum_pool.tile(
                                    [P, NCH], f32, tag="ps", name="ps"
                                )
                                for k in range(KT):
                                    for pi, src in enumerate((xhi, xlo)):
                                        nc.tensor.matmul(
                                            ps[:],
                                            src[k][:, m * P : (m + 1) * P],
                                            wbin_slice(k, n),
                                            start=(k == 0 and pi == 0),
                                            stop=(k == KT - 1 and pi == 1),
                                        )
                                evict(
                                    ps,
                                    m,
                                    n,
                                    use_act=(
                                        np_ == NT // NP - 1
                                        and (mi * NP + ni) % 2 == 1
                                    ),
                                )

        elif mode.startswith("wstat"):
            # W-STATIONARY layout: stationary = sign(W) chunk [128k, 128n],
            # moving = x^T [128k, 512m] f32r, psum = out^T block [128n, 512m].
            #
            # Why: the f32r1 trace shows a 277ns/matmul pace = 512 (moving
            # cols) + ~133 cycles of unhidden LDWEIGHTS - fp32-family
            # stationary operands get no FWL and (4-byte weights) no
            # background weight buffer. sign(W) is {-1,0,+1} = EXACTLY
            # representable in bf16, so a bf16 stationary W restores the
            # hidden-LDWEIGHTS fast path (pace ~= 512 cycles = 220ns) while
            # x stays f32r (fp22) for precision: same error as f32r1.
            # "wstatf32r" keeps W f32r (fallback if mixed dtypes fail).
            f16 = mybir.dt.float16
            wdt = {"wstat": bf16, "wstath": f16}.get(mode, f32r)
            xdt = f16 if mode == "wstath" else f32r
            skip_ldw = mode == "wstatf32rl"
            JT = D_OUT // P  # 16 stationary j-tiles (128 out^T rows each)
            MH = BS // NCH  # 2 moving halves
            xres = [
                res_pool.tile([P, BS], xdt, tag=f"xr{k}", name=f"xr{k}")
                for k in range(KT)
            ]
            wres = [
                res_pool.tile([P, D_OUT], wdt, tag=f"wr{k}", name=f"wr{k}")
                for k in range(KT)
            ]

            # --- load stream, in consumption order, alternating between the
            # sync and scalar HWDGE rings so the ordered stream gets both
            # rings' aggregate bandwidth (~358GB/s HBM cap) instead of one.
            # W chunk (q,k) = sign-source for wres[k][:, q*512:(q+1)*512],
            # 256KB contiguous in the host-relaid wl. Sign ops ride the
            # scalar queue with ~3-chunk lookahead so DMA dispatches stay
            # ahead of the compute waits (no head-of-line starvation).
            rings = [nc.sync, nc.scalar]
            state = {"xring": 1, "wring": 0}
            pend: list = []

            def ring(kind):
                r = rings[state[kind] % 2]
                state[kind] += 1
                return r

            def sign_one():
                q, k, st = pend.pop(0)
                nc.scalar.activation(
                    wres[k][:, q * NCH : (q + 1) * NCH], st[:], Sign
                )

            def load_w(q, k, lag=3, split=False):
                st = wstage.tile([P, NCH], f32, tag="wst", name="wst")
                r0 = (q * KT + k) * P
                if split:
                    # ramp-critical chunks: two contiguous row-halves land on
                    # both rings simultaneously (no per-k arrival skew); the
                    # Sign reads the whole tile so it waits for both.
                    HP = P // 2
                    nc.sync.dma_start(st[:HP, :], w[r0 : r0 + HP, :])
                    nc.scalar.dma_start(st[HP:, :], w[r0 + HP : r0 + P, :])
                else:
                    ring("wring").dma_start(st[:], w[r0 : r0 + P, :])
                pend.append((q, k, st))
                while len(pend) > lag:
                    sign_one()

            xcvt = {}

            def load_x(k, a, b):
                if mode == "wstath":
                    # f32 bytes -> staging -> DVE convert (round-to-nearest)
                    # to the fp16 resident tile. Row-halves ride both rings
                    # (contiguous 256KB each, no skew); the convert waits for
                    # both.
                    if k not in xcvt:
                        xcvt[k] = xstage.tile(
                            [P, BS], f32, tag="xcv", name="xcv", bufs=3
                        )
                    st = xcvt[k]
                    ring("xring").dma_start(st[:, a:b], xT[k * P : (k + 1) * P, a:b])
                    nc.vector.tensor_copy(xres[k][:, a:b], st[:, a:b])
                else:
                    ring("xring").dma_start(
                        xres[k][:, a:b],
                        xT[k * P : (k + 1) * P, a:b].bitcast(f32r),
                    )

            # k=0 split small so the first matmul's deps land fast; the
            # first W chunk leads on the sync ring while x(0) leads on the
            # scalar ring. Sign ops share the scalar queue with half the DMA
            # dispatches, and a Sign waiting on an un-landed chunk blocks
            # every later dispatch behind it (in-order queue) - so signs run
            # with a deep lookahead (lag 6 <= wstage bufs-2) so they only
            # ever touch chunks that have already landed. The first two W
            # chunks use small lags to unblock the first matmuls, emitted
            # after the early x dispatches.
            load_w(0, 0, lag=99)
            load_x(0, 0, NCH)
            sign_one()  # sign(q0,k0): x(0a) dispatch already queued ahead
            load_x(0, NCH, BS)
            for k in range(1, KT):
                # two column-half pieces land on both rings ~simultaneously
                # (row-half splits were tried: a 64-partition DMA only fans
                # out to half the SDMA engines and measured ~18us slower).
                load_x(k, 0, NCH)
                load_x(k, NCH, BS)
                load_w(0, k, lag=1 if k < 4 else 3)
            for q in range(1, 4):
                for k in range(KT):
                    load_w(q, k, lag=3)
            while pend:
                sign_one()

            # --- compute sweeps. First sweep: 8 banks (j0-3 x mh0-1),
            # k-outer so the PE consumes x/W chunks as they land. Later
            # sweeps: 4 banks (j-pair x mh), alternating between the two
            # halves of the PSUM pool so sweep s+2 reuses banks freed by
            # sweep s - eviction (VectorE) fully overlaps the next sweep.
            def mm(ps, k, j, mh, start, stop):
                r = nc.tensor.matmul(
                    ps[:],
                    wres[k][:, j * P : (j + 1) * P],
                    xres[k][:, mh * NCH : (mh + 1) * NCH],
                    start=start,
                    stop=stop,
                )
                if skip_ldw and mh > 0:
                    # Same stationary W[k][j] as the mh-1 matmul: skip the
                    # redundant (unhidden, ~133-cycle for f32r) LDWEIGHTS.
                    r.ins.ldweights = False

            def evict(ps, j, mh, use_act=False, out_ring=None):
                ot = ostage.tile([P, NCH], bf16, tag="ot", name="ot")
                if use_act:
                    nc.scalar.activation(ot[:], ps[:], Sign)
                else:
                    lt = ostage.tile([P, NCH], bf16, tag="lt", name="lt")
                    nc.vector.tensor_scalar(
                        lt[:], ps[:], 0.0, None, mybir.AluOpType.is_lt
                    )
                    nc.vector.scalar_tensor_tensor(
                        ot[:],
                        ps[:],
                        0.0,
                        lt[:],
                        op0=mybir.AluOpType.is_gt,
                        op1=mybir.AluOpType.subtract,
                    )
                (out_ring or nc.gpsimd).dma_start(
                    out[j * P : (j + 1) * P, mh * NCH : (mh + 1) * NCH], ot[:]
                )

            def bank():
                return psum_pool.tile([P, NCH], f32, tag="ps", name="ps")

            banks = [[bank() for _ in range(MH)] for _ in range(4)]
            for k in range(KT):
                for j in range(4):
                    for mh in range(MH):
                        mm(banks[j][mh], k, j, mh, k == 0, k == KT - 1)
            for j in range(4):
                for mh in range(MH):
                    evict(banks[j][mh], j, mh)

            pairs = [(4, 5), (6, 7), (8, 9), (10, 11), (12, 13)]
            for t, jp in enumerate(pairs):
                banks = [[bank() for _ in range(MH)] for _ in range(2)]
                for k in range(KT):
                    for ji in range(2):
                        for mh in range(MH):
                            mm(banks[ji][mh], k, jp[ji], mh, k == 0, k == KT - 1)
                for ji in range(2):
                    for mh in range(MH):
                        # late sweeps: ScalarE (idle by then) takes half the
                        # evictions to shorten the drain tail.
                        evict(
                            banks[ji][mh],
                            jp[ji],
                            mh,
                            use_act=(t >= 3 and (ji * MH + mh) % 2 == 1),
                        )
            # Final sweep (j14,j15): W is fully resident by now, so run
            # bank-serial k-inner - each bank completes every 16 matmuls and
            # its eviction/store overlaps the next bank's matmuls, shrinking
            # the post-last-matmul drain to one bank. Outs ride the scalar
            # HWDGE ring (its loads have long drained; ~0.6us completion vs
            # ~2-3us SWDGE).
            ei = 0
            for j in (14, 15):
                for mh in range(MH):
                    ps = bank()
                    for k in range(KT):
                        mm(ps, k, j, mh, k == 0, k == KT - 1)
                    evict(ps, j, mh, use_act=(ei % 2 == 1), out_ring=nc.scalar)
                    ei += 1

        elif mode == "f32r1":
            NBH = 2  # batch halves (SBUF: W f32 128KB/part + x half 32KB/part)
            BS2 = BS // NBH  # 512
            MT2 = BS2 // P  # 4 m-tiles per half
            xres = [
                res_pool.tile([P, BS2], f32r, tag=f"xr{k}", name=f"xr{k}")
                for k in range(KT)
            ]

            def load_x(k, bh):
                # Direct byte-copy into the f32r tile; the PE truncates fp32
                # to FP22 on read. (A DVE fp32->f32r staging copy was tried:
                # bit-identical flips - DVE truncates too - and it slowed the
                # stream by ~25us. Reverted.)
                nc.sync.dma_start(
                    xres[k][:],
                    xT[k * P : (k + 1) * P, bh * BS2 : (bh + 1) * BS2].bitcast(
                        f32r
                    ),
                )

            for bh in range(NBH):
                for k in range(KT):
                    load_x(k, bh)
                    if bh == 0:
                        # First half: interleave x with the first W halves.
                        load_w_half(k, 0)
                if bh == 0:
                    for h in range(1, NH):
                        for k in range(KT):
                            load_w_half(k, h)

                for n in range(NT):
                    psums = [
                        psum_pool.tile([P, NCH], f32, tag="ps", name="ps")
                        for _ in range(MT2)
                    ]
                    for k in range(KT):
                        for m in range(MT2):
                            nc.tensor.matmul(
                                psums[m][:],
                                xres[k][:, m * P : (m + 1) * P],
                                wbin_slice(k, n),
                                start=(k == 0),
                                stop=(k == KT - 1),
                            )
                    for m in range(MT2):
                        ot = ostage.tile([P, NCH], f32, tag="ot", name="ot")
                        nc.scalar.activation(ot[:], psums[m][:], Sign)
                        nc.sync.dma_start(
                            out[
                                bh * BS2 + m * P : bh * BS2 + (m + 1) * P,
                                n * NCH : (n + 1) * NCH,
                            ],
                            ot[:],
                        )
        else:
            raise ValueError(mode)

    nc.finalize()
    return nc


def _shard_inputs(x: np.ndarray, kernel: np.ndarray, mode: str = MODE):
    """Per-core input maps: batch-shard x (pre-transposed layout), replicate W.

    Pure layout prep only (transpose/reorder of raw f32 bytes) - all
    binarization/compute happens on device.
    """
    if mode.startswith("wstat"):
        # (quarter, k-tile) stream-ordered relayout of the replicated W.
        wl = np.concatenate(
            [
                kernel[k * P : (k + 1) * P, q * NCH : (q + 1) * NCH]
                for q in range(4)
                for k in range(KT)
            ],
            axis=0,
        )
        wl = np.ascontiguousarray(wl)
    else:
        wl = kernel
    in_maps = []
    for i in range(N_CORES):
        xs = np.ascontiguousarray(x[i * BS : (i + 1) * BS, :].T)
        in_maps.append({"xT": xs, "w": wl})
    return in_maps


def run_on_cores(x: np.ndarray, kernel: np.ndarray, mode: str = MODE, **run_kwargs):
    """Compile (cached) and run the SPMD kernel; returns (full_out, BassKernelResults)."""
    from concourse.bass_utils import run_bass_kernel_spmd

    key = ("nc", mode)
    if key not in _CACHE:
        _CACHE[key] = build_bass(mode)
    nc = _CACHE[key]

    in_maps = _shard_inputs(x, kernel, mode)
    res = run_bass_kernel_spmd(nc, in_maps, list(range(N_CORES)), **run_kwargs)
    if mode.startswith("wstat"):
        # Per-core result is out^T [D_OUT, BS] bf16: untranspose + upcast.
        out = np.concatenate(
            [
                np.asarray(res.results[i]["out"]).astype(np.float32).T
                for i in range(N_CORES)
            ],
            axis=0,
        )
    else:
        out = np.concatenate(
            [res.results[i]["out"] for i in range(N_CORES)], axis=0
        )
    return out, res


def kernel(x: np.ndarray, kernel: np.ndarray) -> np.ndarray:
    assert x.shape == (B, D_IN) and kernel.shape == (D_IN, D_OUT)
    out, _ = run_on_cores(
        np.asarray(x, dtype=np.float32), np.asarray(kernel, dtype=np.float32)
    )
    return out.astype(np.float32)



# revision 18
# speedup vs baseline: 1.1613x; 1.0043x over previous
"""Trainium2 Bass kernel for nn_BinaryLayer: out = sign(x @ sign(W)).

x: [8192, 2048] f32, W: [2048, 2048] f32, out: [8192, 2048] f32 (values in {-1,0,1}).

Strategy: data-parallel batch shard across 8 cores (1024 rows each), W
replicated. Host does layout prep only (x shard transpose; W chunk reorder);
all binarization/conversion happens on device.

Default MODE "wstath" (measured 142.6-147us/core across runs - device-level
run-to-run variance of several us was observed, with occasional ~10% slower
outliers under repeated back-to-back runs (thermal/SW throttle); rel err
1.585e-2, 1054/16.7M sign flips - deterministic for the fixed seed-0 inputs,
gate is 2e-2):
  - W-STATIONARY fp16 layout: stationary operand = sign(W) chunk [128k,128n]
    fp16 (+-1/0 are fp16-exact, and 16-bit weights get the FWL fast path +
    background weight buffer, so LDWEIGHTS is fully hidden: measured 216ns
    per N=512 matmul vs 277ns for f32r, whose 4-byte weights can't
    double-buffer). Moving operand = x^T [128k, 512m] fp16 (f32 DMA ->
    VectorE convert; 11-bit mantissa is the sole error source). PSUM banks
    are out^T blocks [128n, 512m]; the host untransposes (free).
  - Everything resident in SBUF (W fp16 64KB/part + x fp16 32KB/part + f32
    staging): no batch halves, W+x each loaded exactly once (24MB/core).
  - Load stream in consumption order, alternating chunks between the sync
    and scalar HWDGE rings (aggregate ~320-375GB/s during the ramp). W
    Sign ops ride the scalar queue with a ~3-chunk lookahead; x converts on
    VectorE. Out^T is written bf16 (sign values exact, half the traffic) via
    the gpsimd SWDGE ring; the final sweep's outs use the scalar ring (fast
    completion) to shrink the drain tail.
  - Compute: first sweep = 8 psum banks (j0-3 x both m-halves), k-outer so
    the PE consumes x/W chunks as they land (the 12MB ramp bounds this
    phase); then 4-bank sweeps alternating psum-pool halves so evictions
    (VectorE (psum>0)-(psum<0) 2-op, or ScalarE Sign) fully overlap the next
    sweep; the last sweep is bank-serial so the post-last-matmul drain is a
    single bank. ~9us framework preamble, ~3us barrier tail.

Other modes kept for reference/fallback:
  "wstatf32r" - same structure, both operands float32r (FP22): ~158us,
            rel err 1.13e-2 (536 flips). The extra ~60 cycles/matmul is
            unhidden f32r LDWEIGHTS.
  "hilo2" - original x-stationary 2-pass bf16 hi/lo, near-fp32-exact
            (1.8e-3), ~250us. Use if the tolerance ever tightens.
  "f32r1" - original x-stationary 1-pass f32r, ~173us, 1.13e-2.
  "wstat" (bf16 W x f32r x) is rejected by walrus ("Mixing of 32-bit and
  non-32-bit Matmult inputs"); "wstatf32rl" (ldweights=False on the second
  matmul of each stationary pair) computes WRONG results - do not use.
"""

import numpy as np

B, D_IN, D_OUT = 8192, 2048, 2048
N_CORES = 8
BS = B // N_CORES  # 1024 batch rows per core
P = 128
KT = D_IN // P  # 16 k-tiles
NCH = 512  # psum bank width (f32)
NT = D_OUT // NCH  # 4 n-chunks

MODE = "wstath"

_CACHE: dict = {}


def build_bass(mode: str = MODE):
    import concourse.mybir as mybir
    import concourse.tile as tile
    from concourse import bacc
    from contextlib import ExitStack

    f32 = mybir.dt.float32
    bf16 = mybir.dt.bfloat16
    f32r = mybir.dt.float32r
    Sign = mybir.ActivationFunctionType.Sign

    # Bacc (not plain Bass): its finalize() runs move_matmul_waits_to_ldweights
    # + generate_event_semaphores, which legalize multi-wait instructions for
    # walrus (each non-event instruction may carry at most one sync wait).
    nc = bacc.Bacc()
    if mode.startswith("wstat"):
        # x^T relaid on host into (k-tile, m-half) blocks: block (k,h) =
        # x^T[k*128:(k+1)*128, h*512:(h+1)*512], 256KB contiguous each, so
        # the two-ring column-half loads are fully sequential reads (the
        # row-major layout gave only 2KB runs -> ~320 vs ~358 GB/s ramp).
        xT = nc.declare_dram_parameter("xT", [2 * D_IN, NCH], f32, isOutput=False)
    else:
        xT = nc.declare_dram_parameter("xT", [D_IN, BS], f32, isOutput=False)
    if mode.startswith("wstat"):
        # W relaid on host into (quarter, k-tile) stream order: chunk (q,k)
        # = W[k*128:(k+1)*128, q*512:(q+1)*512], 256KB contiguous each.
        w = nc.declare_dram_parameter("w", [4 * KT * P, NCH], f32, isOutput=False)
        # out^T in bf16: sign values {-1,0,+1} are bf16-exact; halves the
        # outbound traffic. Host untransposes + converts.
        out = nc.declare_dram_parameter("out", [D_OUT, BS], mybir.dt.bfloat16, isOutput=True)
    else:
        w = nc.declare_dram_parameter("w", [D_IN, D_OUT], f32, isOutput=False)
        out = nc.declare_dram_parameter("out", [BS, D_OUT], f32, isOutput=True)

    with ExitStack() as ctx:
        tc = ctx.enter_context(tile.TileContext(nc))
        res_pool = ctx.enter_context(tc.tile_pool(name="resident", bufs=1))
        xstage = ctx.enter_context(tc.tile_pool(name="xstage", bufs=2))
        # wstatf32r keeps W resident as f32r (128KB/part) - staging pools
        # must shrink to fit the ~208KB/part SBUF budget.
        wstage = ctx.enter_context(
            tc.tile_pool(name="wstage", bufs=3 if mode.startswith("wstatf32r") else 8)
        )
        psum_pool = ctx.enter_context(tc.tile_pool(name="psum", bufs=8, space="PSUM"))
        ostage = ctx.enter_context(
            tc.tile_pool(name="ostage", bufs=3 if mode.startswith("wstatf32r") else 8)
        )

        # W is loaded in half-rows [128, 1024] (4KB contiguous per partition
        # row — 2KB-run column chunks measured only ~225GB/s vs ~300GB/s).
        # f32r note: walrus's verifier requires every writer of an FP32r
        # matmul operand to itself produce float32r, so the f32r tiles are
        # declared f32r, DMAs bitcast the DRAM side (pure byte copy), and the
        # in-place Sign writes f32r (+-1.0 is FP22-exact).
        WH = NCH * 2  # 1024: W half-row width
        NH = D_OUT // WH  # 2 halves
        wdt = bf16 if mode == "hilo2" else f32r
        wbin = [] if mode.startswith("wstat") else [
            [
                res_pool.tile([P, WH], wdt, tag=f"wb{k}_{h}", name=f"wb{k}_{h}")
                for h in range(NH)
            ]
            for k in range(KT)
        ]

        NPH = WH // NCH  # n-chunks per W half

        def wbin_slice(k, n):
            return wbin[k][n // NPH][:, (n % NPH) * NCH : (n % NPH + 1) * NCH]

        def load_w_half(k, h, split=False):
            wsl = w[k * P : (k + 1) * P, h * WH : (h + 1) * WH]
            if mode == "hilo2":
                w32 = xstage.tile([P, WH], f32, tag="w32", name="w32", bufs=3)
                if split:
                    # Two pieces so the first matmul's rhs dep lands sooner.
                    for a, b in ((0, WH // 2), (WH // 2, WH)):
                        nc.sync.dma_start(w32[:, a:b], wsl[:, a:b])
                        nc.scalar.activation(
                            wbin[k][h][:, a:b], w32[:, a:b], Sign
                        )
                else:
                    nc.sync.dma_start(w32[:], wsl)
                    nc.scalar.activation(wbin[k][h][:], w32[:], Sign)
            else:
                # Load into the resident f32r tile and binarize in place.
                nc.sync.dma_start(wbin[k][h][:], wsl.bitcast(f32r))
                nc.scalar.activation(
                    wbin[k][h][:], wbin[k][h][:].bitcast(f32), Sign
                )

        if mode == "hilo2":
            MT = BS // P  # 8 m-tiles
            xhi = [
                res_pool.tile([P, BS], bf16, tag=f"xhi{k}", name=f"xhi{k}")
                for k in range(KT)
            ]
            xlo = [
                res_pool.tile([P, BS], bf16, tag=f"xlo{k}", name=f"xlo{k}")
                for k in range(KT)
            ]

            # Stream: x k-tiles + the first W halves, then the second halves.
            # k=0 is loaded/split in two column pieces so the first matmul's
            # dependencies (xhi[0][:, :128], wbin[0][0][:, :512]) land fast.
            for k in range(KT):
                x32 = xstage.tile([P, BS], f32, tag="x32", name="x32")
                if k == 0 and BS > P:
                    # First-matmul critical path: tiny x piece, then tiny W
                    # piece, before the remainders (queue order = issue order).
                    nc.sync.dma_start(x32[:, :P], xT[0:P, 0:P])
                    nc.vector.tensor_copy(xhi[0][:, :P], x32[:, :P])
                    nc.vector.tensor_sub(xlo[0][:, :P], x32[:, :P], xhi[0][:, :P])
                    load_w_half(k, 0, split=True)
                    nc.sync.dma_start(x32[:, P:], xT[0:P, P:BS])
                    nc.vector.tensor_copy(xhi[0][:, P:], x32[:, P:])
                    nc.vector.tensor_sub(xlo[0][:, P:], x32[:, P:], xhi[0][:, P:])
                else:
                    nc.sync.dma_start(x32[:], xT[k * P : (k + 1) * P, :])
                    nc.vector.tensor_copy(xhi[k][:], x32[:])
                    nc.vector.tensor_sub(xlo[k][:], x32[:], xhi[k][:])
                    load_w_half(k, 0)
            for h in range(1, NH):
                for k in range(KT):
                    load_w_half(k, h)

            # Process n-chunks in pairs (4 m-tiles x 2 n-chunks = 8 PSUM
            # banks): the first pair consumes only W half 0, giving the
            # half-1 DMA stream until ~t=115us to land instead of ~66us.
            # The FIRST sweep is k-outer (consumes W/x k-tiles as they
            # stream); later sweeps are m-outer/k-inner so each PSUM bank
            # completes and evicts individually - the next sweep's matmuls
            # start as soon as a bank frees instead of stalling on a bulk
            # eviction boundary.
            NP = 2  # n-chunks per pair
            MQ = MT // 2  # m-tiles processed per pair sweep (4)

            def evict(psum, m, n, use_act=False):
                # sign(psum) on VectorE as (psum>0) - (psum<0): keeps the
                # eviction off ScalarE, whose in-order queue still holds
                # W-half-1 Sign ops that wait on their DMAs (head-of-line
                # blocking stalled the PE for ~6us at the first sweep edge).
                # The last pair alternates onto ScalarE (idle by then) so the
                # post-last-matmul eviction drain is shorter.
                ot = ostage.tile([P, NCH], f32, tag="ot", name="ot")
                if use_act:
                    nc.scalar.activation(ot[:], psum[:], Sign)
                else:
                    lt = ostage.tile([P, NCH], f32, tag="lt", name="lt")
                    nc.vector.tensor_scalar(
                        lt[:], psum[:], 0.0, None, mybir.AluOpType.is_lt
                    )
                    nc.vector.scalar_tensor_tensor(
                        ot[:],
                        psum[:],
                        0.0,
                        lt[:],
                        op0=mybir.AluOpType.is_gt,
                        op1=mybir.AluOpType.subtract,
                    )
                nc.sync.dma_start(
                    out[m * P : (m + 1) * P, n * NCH : (n + 1) * NCH], ot[:]
                )

            first = True
            for np_ in range(NT // NP):
                for mh in range(2):
                    if first:
                        first = False
                        psums = [
                            [
                                psum_pool.tile([P, NCH], f32, tag="ps", name="ps")
                                for _ in range(NP)
                            ]
                            for _ in range(MQ)
                        ]
                        for k in range(KT):
                            for pi, src in enumerate((xhi, xlo)):
                                for mi in range(MQ):
                                    m = mh * MQ + mi
                                    for ni in range(NP):
                                        nc.tensor.matmul(
                                            psums[mi][ni][:],
                                            src[k][:, m * P : (m + 1) * P],
                                            wbin_slice(k, np_ * NP + ni),
                                            start=(k == 0 and pi == 0),
                                            stop=(k == KT - 1 and pi == 1),
                                        )
                        for mi in range(MQ):
                            for ni in range(NP):
                                evict(
                                    psums[mi][ni],
                                    mh * MQ + mi,
                                    np_ * NP + ni,
                                )
                    else:
                        for mi in range(MQ):
                            m = mh * MQ + mi
                            for ni in range(NP):
                                n = np_ * NP + ni
                                ps = psum_pool.tile(
                                    [P, NCH], f32, tag="ps", name="ps"
                                )
                                for k in range(KT):
                                    for pi, src in enumerate((xhi, xlo)):
                                        nc.tensor.matmul(
                                            ps[:],
                                            src[k][:, m * P : (m + 1) * P],
                                            wbin_slice(k, n),
                                            start=(k == 0 and pi == 0),
                                            stop=(k == KT - 1 and pi == 1),
                                        )
                                evict(
                                    ps,
                                    m,
                                    n,
                                    use_act=(
                                        np_ == NT // NP - 1
                                        and (mi * NP + ni) % 2 == 1
                                    ),
                                )

        elif mode.startswith("wstat"):
            # W-STATIONARY layout: stationary = sign(W) chunk [128k, 128n],
            # moving = x^T [128k, 512m] f32r, psum = out^T block [128n, 512m].
            #
            # Why: the f32r1 trace shows a 277ns/matmul pace = 512 (moving
            # cols) + ~133 cycles of unhidden LDWEIGHTS - fp32-family
            # stationary operands get no FWL and (4-byte weights) no
            # background weight buffer. sign(W) is {-1,0,+1} = EXACTLY
            # representable in bf16, so a bf16 stationary W restores the
            # hidden-LDWEIGHTS fast path (pace ~= 512 cycles = 220ns) while
            # x stays f32r (fp22) for precision: same error as f32r1.
            # "wstatf32r" keeps W f32r (fallback if mixed dtypes fail).
            f16 = mybir.dt.float16
            wdt = {"wstat": bf16, "wstath": f16}.get(mode, f32r)
            xdt = f16 if mode == "wstath" else f32r
            skip_ldw = mode == "wstatf32rl"
            JT = D_OUT // P  # 16 stationary j-tiles (128 out^T rows each)
            MH = BS // NCH  # 2 moving halves
            xres = [
                res_pool.tile([P, BS], xdt, tag=f"xr{k}", name=f"xr{k}")
                for k in range(KT)
            ]
            wres = [
                res_pool.tile([P, D_OUT], wdt, tag=f"wr{k}", name=f"wr{k}")
                for k in range(KT)
            ]

            # --- load stream, in consumption order, alternating between the
            # sync and scalar HWDGE rings so the ordered stream gets both
            # rings' aggregate bandwidth (~358GB/s HBM cap) instead of one.
            # W chunk (q,k) = sign-source for wres[k][:, q*512:(q+1)*512],
            # 256KB contiguous in the host-relaid wl. Sign ops ride the
            # scalar queue with ~3-chunk lookahead so DMA dispatches stay
            # ahead of the compute waits (no head-of-line starvation).
            rings = [nc.sync, nc.scalar]
            state = {"xring": 1, "wring": 0}
            pend: list = []

            def ring(kind):
                r = rings[state[kind] % 2]
                state[kind] += 1
                return r

            def sign_one():
                q, k, st = pend.pop(0)
                nc.scalar.activation(
                    wres[k][:, q * NCH : (q + 1) * NCH], st[:], Sign
                )

            def load_w(q, k, lag=3, split=False):
                st = wstage.tile([P, NCH], f32, tag="wst", name="wst")
                r0 = (q * KT + k) * P
                if split:
                    # ramp-critical chunks: two contiguous row-halves land on
                    # both rings simultaneously (no per-k arrival skew); the
                    # Sign reads the whole tile so it waits for both.
                    HP = P // 2
                    nc.sync.dma_start(st[:HP, :], w[r0 : r0 + HP, :])
                    nc.scalar.dma_start(st[HP:, :], w[r0 + HP : r0 + P, :])
                else:
                    ring("wring").dma_start(st[:], w[r0 : r0 + P, :])
                pend.append((q, k, st))
                while len(pend) > lag:
                    sign_one()

            xcvt = {}

            def load_x(k, a, b):
                if mode == "wstath":
                    # f32 bytes -> staging -> DVE convert (round-to-nearest)
                    # to the fp16 resident tile. Row-halves ride both rings
                    # (contiguous 256KB each, no skew); the convert waits for
                    # both.
                    if k not in xcvt:
                        xcvt[k] = xstage.tile(
                            [P, BS], f32, tag="xcv", name="xcv", bufs=3
                        )
                    st = xcvt[k]
                    blk = 2 * k + a // NCH
                    ring("xring").dma_start(
                        st[:, a:b], xT[blk * P : (blk + 1) * P, :]
                    )
                    nc.vector.tensor_copy(xres[k][:, a:b], st[:, a:b])
                else:
                    blk = 2 * k + a // NCH
                    ring("xring").dma_start(
                        xres[k][:, a:b],
                        xT[blk * P : (blk + 1) * P, :].bitcast(f32r),
                    )

            # k=0 split small so the first matmul's deps land fast; the
            # first W chunk leads on the sync ring while x(0) leads on the
            # scalar ring. Sign ops share the scalar queue with half the DMA
            # dispatches, and a Sign waiting on an un-landed chunk blocks
            # every later dispatch behind it (in-order queue) - so signs run
            # with a deep lookahead (lag 6 <= wstage bufs-2) so they only
            # ever touch chunks that have already landed. The first two W
            # chunks use small lags to unblock the first matmuls, emitted
            # after the early x dispatches.
            load_w(0, 0, lag=99)
            load_x(0, 0, NCH)
            sign_one()  # sign(q0,k0): x(0a) dispatch already queued ahead
            load_x(0, NCH, BS)
            for k in range(1, KT):
                # two column-half pieces land on both rings ~simultaneously
                # (row-half splits were tried: a 64-partition DMA only fans
                # out to half the SDMA engines and measured ~18us slower).
                load_x(k, 0, NCH)
                load_x(k, NCH, BS)
                load_w(0, k, lag=1 if k < 4 else 3)
            for q in range(1, 4):
                for k in range(KT):
                    load_w(q, k, lag=3)
            while pend:
                sign_one()

            # --- compute sweeps. First sweep: 8 banks (j0-3 x mh0-1),
            # k-outer so the PE consumes x/W chunks as they land. Later
            # sweeps: 4 banks (j-pair x mh), alternating between the two
            # halves of the PSUM pool so sweep s+2 reuses banks freed by
            # sweep s - eviction (VectorE) fully overlaps the next sweep.
            def mm(ps, k, j, mh, start, stop):
                r = nc.tensor.matmul(
                    ps[:],
                    wres[k][:, j * P : (j + 1) * P],
                    xres[k][:, mh * NCH : (mh + 1) * NCH],
                    start=start,
                    stop=stop,
                )
                if skip_ldw and mh > 0:
                    # Same stationary W[k][j] as the mh-1 matmul: skip the
                    # redundant (unhidden, ~133-cycle for f32r) LDWEIGHTS.
                    r.ins.ldweights = False

            def evict(ps, j, mh, use_act=False, out_ring=None):
                ot = ostage.tile([P, NCH], bf16, tag="ot", name="ot")
                if use_act:
                    nc.scalar.activation(ot[:], ps[:], Sign)
                else:
                    lt = ostage.tile([P, NCH], bf16, tag="lt", name="lt")
                    nc.vector.tensor_scalar(
                        lt[:], ps[:], 0.0, None, mybir.AluOpType.is_lt
                    )
                    nc.vector.scalar_tensor_tensor(
                        ot[:],
                        ps[:],
                        0.0,
                        lt[:],
                        op0=mybir.AluOpType.is_gt,
                        op1=mybir.AluOpType.subtract,
                    )
                (out_ring or nc.gpsimd).dma_start(
                    out[j * P : (j + 1) * P, mh * NCH : (mh + 1) * NCH], ot[:]
                )

            def bank():
                return psum_pool.tile([P, NCH], f32, tag="ps", name="ps")

            banks = [[bank() for _ in range(MH)] for _ in range(4)]
            for k in range(KT):
                for j in range(4):
                    for mh in range(MH):
                        mm(banks[j][mh], k, j, mh, k == 0, k == KT - 1)
            for j in range(4):
                for mh in range(MH):
                    evict(banks[j][mh], j, mh)

            pairs = [(4, 5), (6, 7), (8, 9), (10, 11), (12, 13)]
            for t, jp in enumerate(pairs):
                banks = [[bank() for _ in range(MH)] for _ in range(2)]
                for k in range(KT):
                    for ji in range(2):
                        for mh in range(MH):
                            mm(banks[ji][mh], k, jp[ji], mh, k == 0, k == KT - 1)
                for ji in range(2):
                    for mh in range(MH):
                        # late sweeps: ScalarE (idle by then) takes half the
                        # evictions to shorten the drain tail.
                        evict(
                            banks[ji][mh],
                            jp[ji],
                            mh,
                            use_act=(t >= 3 and (ji * MH + mh) % 2 == 1),
                        )
            # Final sweep (j14,j15): W is fully resident by now, so run
            # bank-serial k-inner - each bank completes every 16 matmuls and
            # its eviction/store overlaps the next bank's matmuls, shrinking
            # the post-last-matmul drain to one bank. Outs ride the scalar
            # HWDGE ring (its loads have long drained; ~0.6us completion vs
            # ~2-3us SWDGE).
            ei = 0
            for j in (14, 15):
                for mh in range(MH):
                    ps = bank()
                    for k in range(KT):
                        mm(ps, k, j, mh, k == 0, k == KT - 1)
                    evict(ps, j, mh, use_act=(ei % 2 == 1), out_ring=nc.scalar)
                    ei += 1

        elif mode == "f32r1":
            NBH = 2  # batch halves (SBUF: W f32 128KB/part + x half 32KB/part)
            BS2 = BS // NBH  # 512
            MT2 = BS2 // P  # 4 m-tiles per half
            xres = [
                res_pool.tile([P, BS2], f32r, tag=f"xr{k}", name=f"xr{k}")
                for k in range(KT)
            ]

            def load_x(k, bh):
                # Direct byte-copy into the f32r tile; the PE truncates fp32
                # to FP22 on read. (A DVE fp32->f32r staging copy was tried:
                # bit-identical flips - DVE truncates too - and it slowed the
                # stream by ~25us. Reverted.)
                nc.sync.dma_start(
                    xres[k][:],
                    xT[k * P : (k + 1) * P, bh * BS2 : (bh + 1) * BS2].bitcast(
                        f32r
                    ),
                )

            for bh in range(NBH):
                for k in range(KT):
                    load_x(k, bh)
                    if bh == 0:
                        # First half: interleave x with the first W halves.
                        load_w_half(k, 0)
                if bh == 0:
                    for h in range(1, NH):
                        for k in range(KT):
                            load_w_half(k, h)

                for n in range(NT):
                    psums = [
                        psum_pool.tile([P, NCH], f32, tag="ps", name="ps")
                        for _ in range(MT2)
                    ]
                    for k in range(KT):
                        for m in range(MT2):
                            nc.tensor.matmul(
                                psums[m][:],
                                xres[k][:, m * P : (m + 1) * P],
                                wbin_slice(k, n),
                                start=(k == 0),
                                stop=(k == KT - 1),
                            )
                    for m in range(MT2):
                        ot = ostage.tile([P, NCH], f32, tag="ot", name="ot")
                        nc.scalar.activation(ot[:], psums[m][:], Sign)
                        nc.sync.dma_start(
                            out[
                                bh * BS2 + m * P : bh * BS2 + (m + 1) * P,
                                n * NCH : (n + 1) * NCH,
                            ],
                            ot[:],
                        )
        else:
            raise ValueError(mode)

    nc.finalize()
    return nc


def _shard_inputs(x: np.ndarray, kernel: np.ndarray, mode: str = MODE):
    """Per-core input maps: batch-shard x (pre-transposed layout), replicate W.

    Pure layout prep only (transpose/reorder of raw f32 bytes) - all
    binarization/compute happens on device.
    """
    if mode.startswith("wstat"):
        # (quarter, k-tile) stream-ordered relayout of the replicated W.
        wl = np.concatenate(
            [
                kernel[k * P : (k + 1) * P, q * NCH : (q + 1) * NCH]
                for q in range(4)
                for k in range(KT)
            ],
            axis=0,
        )
        wl = np.ascontiguousarray(wl)
    else:
        wl = kernel
    in_maps = []
    for i in range(N_CORES):
        xs = np.ascontiguousarray(x[i * BS : (i + 1) * BS, :].T)
        if mode.startswith("wstat"):
            # (k-tile, m-half) block relayout matching the device's
            # contiguous-chunk load stream.
            xs = np.ascontiguousarray(
                xs.reshape(KT, P, 2, NCH).swapaxes(1, 2).reshape(2 * D_IN, NCH)
            )
        in_maps.append({"xT": xs, "w": wl})
    return in_maps


def run_on_cores(x: np.ndarray, kernel: np.ndarray, mode: str = MODE, **run_kwargs):
    """Compile (cached) and run the SPMD kernel; returns (full_out, BassKernelResults)."""
    from concourse.bass_utils import run_bass_kernel_spmd

    key = ("nc", mode)
    if key not in _CACHE:
        _CACHE[key] = build_bass(mode)
    nc = _CACHE[key]

    in_maps = _shard_inputs(x, kernel, mode)
    res = run_bass_kernel_spmd(nc, in_maps, list(range(N_CORES)), **run_kwargs)
    if mode.startswith("wstat"):
        # Per-core result is out^T [D_OUT, BS] bf16: untranspose + upcast.
        out = np.concatenate(
            [
                np.asarray(res.results[i]["out"]).astype(np.float32).T
                for i in range(N_CORES)
            ],
            axis=0,
        )
    else:
        out = np.concatenate(
            [res.results[i]["out"] for i in range(N_CORES)], axis=0
        )
    return out, res


def kernel(x: np.ndarray, kernel: np.ndarray) -> np.ndarray:
    assert x.shape == (B, D_IN) and kernel.shape == (D_IN, D_OUT)
    out, _ = run_on_cores(
        np.asarray(x, dtype=np.float32), np.asarray(kernel, dtype=np.float32)
    )
    return out.astype(np.float32)

